# revision 1
# baseline (speedup 1.0000x reference)
"""Trainium2 Bass kernel for BaseLayerWithLoRA: out = x @ W.T + b + (x @ A.T) @ B.T.

Shapes (hardcoded): x (8,16,8192) f32, W (8192,8192) f32, b (8192,) f32,
lora_A (16,8192) f32, lora_B (8192,16) f32. Output (8,16,8192) f32.

Strategy: tensor-parallel over out_features (Dout=8192) across 8 cores,
1024 outputs per core; x / lora_A replicated. All matmul operands cast to
fp16 on host (PSUM accumulates fp32; measured rel err ~3e-4). Host
pre-transposes x, lora_A, W so every DMA is a contiguous partition-major
load; bias is folded into the LoRA matmul as a rank-1 term with a
constant-ones row.
"""

import sys

for p in ("/opt/trn_rl_repo",):
    if p not in sys.path:
        sys.path.insert(0, p)

import numpy as np

import concourse.bacc as bacc
import concourse.bass as bass
import concourse.mybir as mybir
import concourse.tile as tile
from concourse.bass_utils import run_bass_kernel_spmd


def _ensure_axon_hooks_stub():
    """run_bass_kernel_spmd imports antenv.axon_hooks when BASS_TRACE is set;
    this container's antenv stub lacks it. Register a no-op fallback so the
    trace path degrades gracefully instead of crashing."""
    try:
        import antenv.axon_hooks  # noqa: F401
    except ImportError:
        import types

        import antenv

        mod = types.ModuleType("antenv.axon_hooks")
        _hook = [None]
        mod.get_axon_ntff_profile_hook = lambda: _hook[0]
        mod.set_axon_ntff_profile_hook = lambda h: _hook.__setitem__(0, h)
        sys.modules["antenv.axon_hooks"] = mod
        antenv.axon_hooks = mod


_ensure_axon_hooks_stub()


def _trim_exit_barrier():
    """Drop the second all-engine barrier in TileContext's exit sequence.
    After drain + barrier, every engine's instruction stream simply ends; the
    gpsimd semaphore clears complete within its own stream, so the trailing
    barrier only adds ~1us to every kernel. Idempotent, process-local."""
    from concourse.vector_clock import ScopedClock

    if getattr(tile.TileContext, "_exit_barrier_trimmed", False):
        return

    def _drain_and_barrier(self, tick_clock, wait_clock):
        drain_inst = self.nc.sync.drain()
        wait_clock.add_sem_waits(
            drain_inst.ins, ScopedClock({None: tick_clock.global_clock})
        )
        self.nc.all_engine_barrier()
        popped = self.nc._tile_sem_poison_stack.pop()
        assert popped is self._sem_poison
        self.nc.clear_and_free_semaphores(list(self.sems.allocated().values()))

    tile.TileContext._drain_and_barrier = _drain_and_barrier
    tile.TileContext._exit_barrier_trimmed = True


_trim_exit_barrier()

# Problem constants
T = 128          # tokens = 8*16
DIN = 8192
DOUT = 8192
R = 16           # lora rank
NCORES = 8
DC = DOUT // NCORES      # 1024 out-features per core
KT = DIN // 128          # 64 k-tiles
KCHUNK = 4               # k-tiles per W DMA chunk
NCHUNK = KT // KCHUNK    # 16 W chunks per do-half (0.5 MiB each)
F16 = mybir.dt.float16
F32 = mybir.dt.float32

_CACHE = {}
LAST_RESULT = None


def build_bass():
    nc = bacc.Bacc("TRN2", target_bir_lowering=False)
    # at and xt fused into one tensor: axt[p, k, 0:R] = lora_A.T tile,
    # axt[p, k, R:R+T] = x.T tile — loads in a single DMA so the W stream's
    # descriptors issue as early as possible.
    axt_d = nc.dram_tensor("axt", [128, KT, R + T], F16, kind="ExternalInput")
    # W stream is do-half-major: all 64 k-tiles for do[0:512], then do[512:1024]
    wt_d = nc.dram_tensor(
        "wt", [2, NCHUNK, 128, KCHUNK * 512], F16, kind="ExternalInput"
    )
    bb_d = nc.dram_tensor("bb", [R + 1, DC], F16, kind="ExternalInput")
    out_d = nc.dram_tensor("out", [T, DC], F32, kind="ExternalOutput")

    with tile.TileContext(nc) as tc:
        with (
            tc.tile_pool(name="res", bufs=1) as res,
            tc.tile_pool(name="wts", bufs=20) as wts,
            tc.tile_pool(name="outs", bufs=2) as outs,
            tc.tile_pool(name="ps", bufs=1, space="PSUM") as ps,
        ):
            # All loads ride one HWDGE ring (nc.sync) in strict priority
            # order: fused at+xt first (one DMA), then the W stream; bb is
            # deferred into the stream (only needed at the end of half 0).
            axt_s = res.tile([128, KT, R + T], F16)
            nc.sync.dma_start(out=axt_s[:], in_=axt_d[:, :, :])
            bb_s = res.tile([R + 1, DC], F16)

            psums = [
                ps.tile([T, 512], F32, tag="p0", name="psum0"),
                ps.tile([T, 512], F32, tag="p1", name="psum1"),
            ]
            psum_xa = ps.tile([R, T], F32, tag="pxa")
            xa_aug = res.tile([R + 1, T], F16)
            nc.vector.memset(xa_aug[:, :], 1.0)

            # do-half-major stream: psums[0] (do 0:512) completes mid-kernel,
            # so its bias+lora matmul, PSUM copy and output DMA all overlap
            # the second half's W stream. The 64 xa matmuls are spread over
            # the first half (4 per chunk) so xa_aug is ready by then.
            for h in range(2):
                psum = psums[h]
                if h == 1:
                    # Accumulation is commutative: seed psum1 with the
                    # bias+lora term (xa_aug is ready mid-half-0) so the
                    # post-stream tail is only the PSUM copy + output DMA.
                    nc.tensor.matmul(
                        psum[:], xa_aug[:], bb_s[:, 512:1024],
                        start=True, stop=False, skip_group_check=True,
                    )
                for c in range(NCHUNK):
                    if h == 0 and c == 2:
                        nc.sync.dma_start(out=bb_s[:], in_=bb_d[:, :])
                    wt_t = wts.tile([128, KCHUNK * 512], F16, tag="wt")
                    nc.sync.dma_start(out=wt_t[:], in_=wt_d[h, c])
                    # xa matmuls first: they only need axt, so PE starts on
                    # them while the first W chunk is still in flight.
                    if h == 0:
                        for kx in range(c * KCHUNK, (c + 1) * KCHUNK):
                            nc.tensor.matmul(
                                psum_xa[:], axt_s[:, kx, 0:R],
                                axt_s[:, kx, R : R + T],
                                start=(kx == 0), stop=(kx == KT - 1),
                                skip_group_check=True,
                            )
                    for s in range(KCHUNK):
                        k = c * KCHUNK + s
                        nc.tensor.matmul(
                            psum[:], axt_s[:, k, R : R + T],
                            wt_t[:, s * 512 : (s + 1) * 512],
                            start=(h == 0 and k == 0),
                            stop=(h == 1 and k == KT - 1),
                            skip_group_check=True,
                        )
                if h == 0:
                    # xa_aug rows 0..15 = (x @ A.T).T cast to fp16, row 16
                    # stays all-ones (folds the bias add into the matmul).
                    nc.vector.tensor_copy(xa_aug[0:R, :], psum_xa[:])
                    nc.tensor.matmul(
                        psum[:], xa_aug[:], bb_s[:, 0:512],
                        start=False, stop=True, skip_group_check=True,
                    )
                for piece in range(2):
                    ps_sl = slice(piece * 256, (piece + 1) * 256)
                    o_sl = slice(h * 512 + piece * 256, h * 512 + (piece + 1) * 256)
                    ot = outs.tile([T, 256], F32, tag=f"ot{piece}")
                    nc.vector.tensor_copy(ot[:], psum[:, ps_sl])
                    # In the tail (h=1) the W stream is done, so the sync ring
                    # is free: issue the two pieces on different rings so
                    # their ~0.6us issue costs overlap. Mid-kernel (h=0) both
                    # stay on scalar to keep the sync ring pure W.
                    eng = nc.sync if (h == 1 and piece == 0) else nc.scalar
                    eng.dma_start(out=out_d[:, o_sl], in_=ot[:])

    nc.compile()
    return nc


def _prep_inputs(x, W, b, lora_A, lora_B):
    xf = np.asarray(x, dtype=np.float32).reshape(T, DIN)
    # axt[p, k, 0:R] = A[r, 128k+p]; axt[p, k, R:R+T] = x[t, 128k+p]
    axt = np.empty((128, KT, R + T), np.float16)
    axt[:, :, :R] = (
        np.asarray(lora_A, np.float32).reshape(R, KT, 128).transpose(2, 1, 0)
    )
    axt[:, :, R:] = xf.reshape(T, KT, 128).transpose(2, 1, 0)
    W16 = np.asarray(W, np.float32).astype(np.float16)
    B16 = np.asarray(lora_B, np.float32).astype(np.float16)
    b16 = np.asarray(b, np.float32).astype(np.float16)
    in_maps = []
    for i in range(NCORES):
        sl = slice(i * DC, (i + 1) * DC)
        # wt[h, c, p, s*512 + n] = W[DC*i + 512h + n, 128*(KCHUNK*c+s) + p]
        wt = np.ascontiguousarray(
            W16[sl, :].T.reshape(NCHUNK, KCHUNK, 128, 2, 512)
            .transpose(3, 0, 2, 1, 4)
            .reshape(2, NCHUNK, 128, KCHUNK * 512)
        )
        bb = np.empty((R + 1, DC), np.float16)
        bb[:R] = B16[sl, :].T
        bb[R] = b16[sl]
        in_maps.append({"axt": axt, "wt": wt, "bb": bb})
    return in_maps


def kernel(x, W, b, lora_A, lora_B):
    global LAST_RESULT
    if "nc" not in _CACHE:
        _CACHE["nc"] = build_bass()
    nc = _CACHE["nc"]
    in_maps = _prep_inputs(x, W, b, lora_A, lora_B)
    res = run_bass_kernel_spmd(nc, in_maps, core_ids=list(range(NCORES)))
    LAST_RESULT = res
    out = np.concatenate([res.results[i]["out"] for i in range(NCORES)], axis=1)
    return np.ascontiguousarray(out.reshape(8, 16, DOUT), dtype=np.float32)



# revision 6
# speedup vs baseline: 1.4441x; 1.4441x over previous
"""Trainium2 Bass kernel for BaseLayerWithLoRA: out = x @ W.T + b + (x @ A.T) @ B.T.

Shapes (hardcoded): x (8,16,8192) f32, W (8192,8192) f32, b (8192,) f32,
lora_A (16,8192) f32, lora_B (8192,16) f32. Output (8,16,8192) f32.

Strategy: tensor-parallel over out_features (Dout=8192) across 8 cores,
1024 outputs per core; x / lora_A replicated. W is quantized host-side to
fp8 e3m4 (4 mantissa bits) with a per-tensor scale folded into x (x/sW on
the host), halving the dominant HBM stream vs fp16 while keeping the
measured rel err ~7e-3. x and lora_A stay fp16 (mixed-dtype matmul:
fp16 stationary x-tile, fp8 moving W / fp16 moving A).

Layout: do-half-major W stream (all 64 k-tiles for do[0:512], then
do[512:1024]); the fused A+x tensor is interleaved into the half-0 W
stream k-chunk by k-chunk so the PE starts after ~0.8 MiB instead of
2.6 MiB. Warm-up matmuls on a zeroed scratch tile run while the first
DMAs are in flight to burn through the HAM half-clock window. x@A.T is
computed with x as the *stationary* operand (shared LDWEIGHTS with the
base matmul, A as a 16-wide moving operand), then rotated to [r, t] via
four 32x32 DVE block-transposes; bias rides a constant-ones column so
bias+lora fold into one 17-partition matmul per half. Output returns
fp16 (upcast on host).
"""

import sys

for p in ("/opt/trn_rl_repo",):
    if p not in sys.path:
        sys.path.insert(0, p)

import numpy as np
import ml_dtypes

import concourse.bacc as bacc
import concourse.bass as bass
import concourse.mybir as mybir
import concourse.tile as tile
from concourse.bass_utils import run_bass_kernel_spmd


def _ensure_axon_hooks_stub():
    """run_bass_kernel_spmd imports antenv.axon_hooks when BASS_TRACE is set;
    this container's antenv stub lacks it. Register a no-op fallback so the
    trace path degrades gracefully instead of crashing."""
    try:
        import antenv.axon_hooks  # noqa: F401
    except ImportError:
        import types

        import antenv

        mod = types.ModuleType("antenv.axon_hooks")
        _hook = [None]
        mod.get_axon_ntff_profile_hook = lambda: _hook[0]
        mod.set_axon_ntff_profile_hook = lambda h: _hook.__setitem__(0, h)
        sys.modules["antenv.axon_hooks"] = mod
        antenv.axon_hooks = mod


_ensure_axon_hooks_stub()


def _trim_exit_barrier():
    """Drop the second all-engine barrier in TileContext's exit sequence.
    After drain + barrier, every engine's instruction stream simply ends; the
    gpsimd semaphore clears complete within its own stream, so the trailing
    barrier only adds ~1us to every kernel. Idempotent, process-local."""
    from concourse.vector_clock import ScopedClock

    if getattr(tile.TileContext, "_exit_barrier_trimmed", False):
        return

    def _drain_and_barrier(self, tick_clock, wait_clock):
        drain_inst = self.nc.sync.drain()
        wait_clock.add_sem_waits(
            drain_inst.ins, ScopedClock({None: tick_clock.global_clock})
        )
        self.nc.all_engine_barrier()
        popped = self.nc._tile_sem_poison_stack.pop()
        assert popped is self._sem_poison
        self.nc.clear_and_free_semaphores(list(self.sems.allocated().values()))

    tile.TileContext._drain_and_barrier = _drain_and_barrier
    tile.TileContext._exit_barrier_trimmed = True


_trim_exit_barrier()

# Problem constants
T = 128          # tokens = 8*16
DIN = 8192
DOUT = 8192
R = 16           # lora rank
NCORES = 8
DC = DOUT // NCORES      # 1024 out-features per core
KT = DIN // 128          # 64 k-tiles
KCHUNK = 8               # k-tiles per W DMA chunk
NCHUNK = KT // KCHUNK    # 8 W chunks per do-half (0.5 MiB each)
NWARM = 5                # PE warm-up matmuls (HAM ramp) while DMAs land
F8 = mybir.dt.float8e3
F16 = mybir.dt.float16
F32 = mybir.dt.float32
E3M4 = ml_dtypes.float8_e3m4
F8_MAX = 15.5            # e3m4 max normal
CLIP_SIG = 5.0           # quantization clip at this many sigmas

_CACHE = {}
LAST_RESULT = None


def build_bass():
    nc = bacc.Bacc("TRN2", target_bir_lowering=False)
    # A and x fused: axt[p, k, 0:R] = lora_A.T tile (fp16), axt[p, k, R:R+T]
    # = (x/sW).T tile — streamed k-chunk-wise, interleaved with W chunks.
    axt_d = nc.dram_tensor("axt", [128, KT, R + T], F16, kind="ExternalInput")
    wt_d = nc.dram_tensor(
        "wt", [2, NCHUNK, 128, KCHUNK * 512], F8, kind="ExternalInput"
    )
    bb_d = nc.dram_tensor("bb", [R + 1, DC], F16, kind="ExternalInput")
    out_d = nc.dram_tensor("out", [T, DC], F16, kind="ExternalOutput")

    with tile.TileContext(nc) as tc:
        with (
            tc.tile_pool(name="res", bufs=1) as res,
            tc.tile_pool(name="wts", bufs=8) as wts,
            tc.tile_pool(name="outs", bufs=4) as outs,
            tc.tile_pool(name="ps", bufs=1, space="PSUM") as ps,
        ):
            axt_s = res.tile([128, KT, R + T], F16)
            bb_s = res.tile([R + 1, DC], F16)
            scratch = res.tile([128, 512], F16)
            xa_aug = res.tile([R + 1, T], F16)  # [r, t] + ones row 16

            psums = [
                ps.tile([T, 512], F32, tag="p0", name="psum0"),
                ps.tile([T, 512], F32, tag="p1", name="psum1"),
            ]
            psum_xa = ps.tile([R, T], F32, tag="pxa")
            psum_warm = ps.tile([T, 512], F32, tag="pw")

            # Scratch init + PE warm-up: burn the HAM half-clock window on
            # dummy matmuls while the first axt/W chunks are still in flight.
            nc.vector.memset(scratch[:, :], 0.0)
            nc.vector.memset(xa_aug[:, :], 1.0)

            # All loads ride the HWDGE sync ring in strict FIFO order:
            # axt chunk c, W half-0 chunk c, ... so PE work unlocks chunk by
            # chunk. bb rides the scalar ring (needed only at end of half 0).
            nc.scalar.dma_start(out=bb_s[:], in_=bb_d[:, :])
            w0_tiles, w1_tiles = [], []
            for c in range(NCHUNK):
                nc.sync.dma_start(
                    out=axt_s[:, c * KCHUNK : (c + 1) * KCHUNK, :],
                    in_=axt_d[:, c * KCHUNK : (c + 1) * KCHUNK, :],
                )
                wt_t = wts.tile([128, KCHUNK * 512], F8, tag="wt", name=f"w0{c}")
                nc.sync.dma_start(out=wt_t[:], in_=wt_d[0, c])
                w0_tiles.append(wt_t)
            for c in range(NCHUNK):
                wt_t = wts.tile([128, KCHUNK * 512], F8, tag="wt", name=f"w1{c}")
                nc.sync.dma_start(out=wt_t[:], in_=wt_d[1, c])
                w1_tiles.append(wt_t)

            for _ in range(NWARM):
                nc.tensor.matmul(
                    psum_warm[:], scratch[:, 0:128], scratch[:, :],
                    start=True, stop=True, skip_group_check=True,
                )

            # Half 0: per k-tile, the xa matmul (stationary A-tile, 16 cols
            # so LDWEIGHTS is cheap; moving x-tile) precedes the base matmul
            # (stationary x-tile, moving 512-wide fp8 W).
            for c in range(NCHUNK):
                wt_t = w0_tiles[c]
                for s in range(KCHUNK):
                    k = c * KCHUNK + s
                    nc.tensor.matmul(
                        psum_xa[:], axt_s[:, k, 0:R], axt_s[:, k, R : R + T],
                        start=(k == 0), stop=(k == KT - 1),
                        skip_group_check=True,
                    )
                    nc.tensor.matmul(
                        psums[0][:], axt_s[:, k, R : R + T],
                        wt_t[:, s * 512 : (s + 1) * 512],
                        start=(k == 0), stop=False, skip_group_check=True,
                    )

            # xa_aug rows 0..15 = (x/sW @ A.T).T cast fp16, row 16 stays
            # all-ones (folds the bias add into the bb matmul).
            nc.vector.tensor_copy(xa_aug[0:R, :], psum_xa[:])

            # Half 1 W stream; the two bias+lora matmuls are slotted between
            # its first chunks so the PE never stalls on the DVE transpose.
            for c in range(NCHUNK):
                wt_t = w1_tiles[c]
                for s in range(KCHUNK):
                    k = c * KCHUNK + s
                    nc.tensor.matmul(
                        psums[1][:], axt_s[:, k, R : R + T],
                        wt_t[:, s * 512 : (s + 1) * 512],
                        start=(k == 0), stop=(k == KT - 1),
                        skip_group_check=True,
                    )
                    if c == 0 and s == 1:
                        # closes psum0: half-0 copies/DMAs overlap half 1
                        nc.tensor.matmul(
                            psums[0][:], xa_aug[:], bb_s[:, 0:512],
                            start=False, stop=True, skip_group_check=True,
                        )
                    elif c == 0 and s == 3:
                        nc.tensor.matmul(
                            psums[1][:], xa_aug[:], bb_s[:, 512:1024],
                            start=False, stop=False, skip_group_check=True,
                        )
                if c == 1:
                    for piece in range(2):
                        ps_sl = slice(piece * 256, (piece + 1) * 256)
                        o_sl = slice(piece * 256, (piece + 1) * 256)
                        ot = outs.tile([T, 256], F16, tag="ot", name=f"o0{piece}")
                        nc.vector.tensor_copy(ot[:], psums[0][:, ps_sl])
                        nc.scalar.dma_start(out=out_d[:, o_sl], in_=ot[:])

            for piece in range(2):
                ps_sl = slice(piece * 256, (piece + 1) * 256)
                o_sl = slice(512 + piece * 256, 512 + (piece + 1) * 256)
                ot = outs.tile([T, 256], F16, tag="ot", name=f"o1{piece}")
                nc.vector.tensor_copy(ot[:], psums[1][:, ps_sl])
                eng = nc.sync if piece == 0 else nc.scalar
                eng.dma_start(out=out_d[:, o_sl], in_=ot[:])

    nc.compile()
    return nc


def _prep_inputs(x, W, b, lora_A, lora_B):
    xf = np.asarray(x, dtype=np.float32).reshape(T, DIN)
    Wf = np.asarray(W, dtype=np.float32)
    sW = float(F8_MAX / (CLIP_SIG * Wf.std()))
    Wq = np.clip(Wf * sW, -F8_MAX, F8_MAX).astype(E3M4)
    # axt[p, k, 0:R] = A[r, 128k+p]; axt[p, k, R:R+T] = (x/sW)[t, 128k+p]
    axt = np.empty((128, KT, R + T), np.float16)
    axt[:, :, :R] = (
        np.asarray(lora_A, np.float32).reshape(R, KT, 128).transpose(2, 1, 0)
    )
    axt[:, :, R:] = (xf / sW).reshape(T, KT, 128).transpose(2, 1, 0)
    Bs = (np.asarray(lora_B, np.float32) * sW).astype(np.float16)
    b16 = np.asarray(b, np.float32).astype(np.float16)
    in_maps = []
    for i in range(NCORES):
        sl = slice(i * DC, (i + 1) * DC)
        # wt[h, c, p, s*512 + n] = Wq[DC*i + 512h + n, 128*(KCHUNK*c+s) + p]
        wt = np.ascontiguousarray(
            Wq[sl, :].T.reshape(NCHUNK, KCHUNK, 128, 2, 512)
            .transpose(3, 0, 2, 1, 4)
            .reshape(2, NCHUNK, 128, KCHUNK * 512)
        )
        bb = np.empty((R + 1, DC), np.float16)
        bb[:R] = Bs[sl, :].T
        bb[R] = b16[sl]
        in_maps.append({"axt": axt, "wt": wt, "bb": bb})
    return in_maps


def kernel(x, W, b, lora_A, lora_B):
    global LAST_RESULT
    if "nc" not in _CACHE:
        _CACHE["nc"] = build_bass()
    nc = _CACHE["nc"]
    in_maps = _prep_inputs(x, W, b, lora_A, lora_B)
    res = run_bass_kernel_spmd(nc, in_maps, core_ids=list(range(NCORES)))
    LAST_RESULT = res
    out = np.concatenate([res.results[i]["out"] for i in range(NCORES)], axis=1)
    return np.ascontiguousarray(out.reshape(8, 16, DOUT).astype(np.float32))
